# revision 33
# baseline (speedup 1.0000x reference)
"""Trainium2 Bass kernel for Luong local-p sparse attention.

Math (per batch n, full shapes N=64, L=258, H=1024, Q=256):
    score = (h_t @ W_a) @ enc^T           masked to window [p_t-16, p_t+16]
    align = softmax(score) * gauss(p_t)
    out   = tanh([align @ enc, h_t] @ W_c^T)

Only a 33-wide window of enc survives the mask, so the kernel gathers
windows host-side and pushes W_a / W_c[:, :H] through the 33-wide side:
    u  = W_a-transform of window   (uT[h', (n,j)] = sum_h W_aT[h,h'] enc_w[(n,j),h])
    s  = uT^T-partial scores       (score^T[j, q] = sum_h' uT[h',j] h_t[q,h'])
    softmax over j (33 rows) j-major with a 4th-power renormalization
    v  = W_c1-transform of window  (v[(n,j), h'] = sum_h enc_w[(n,j),h] W_c1T[h,h'])
    outT[h', (n,q)] = tanh(sum_h W_c2T[h,h'] dec[h,(n,q)] + sum_j v[j,h'] t[j,q])

The output GEMM runs TRANSPOSED (outT layout [H, B*Q]): stationary W_c2
chunks are shared across batches, the per-batch ctx matmuls (contraction
over the 33 window rows) accumulate into the same PSUM tiles, and tanh +
stores pipeline per 128-row output chunk (no big tail).

DMA priority order: enc -> W_a -> W_c1 -> dec b0,b1 -> W_c2 -> dec b2..b7,
all on the sync ring (queue 1 alone sustains ~390 GB/s; the 8 HWDGE
completion-semaphore lanes are shared sync<->scalar, so v-window scatters
ride the gpsimd ring's disjoint lanes and output stores queue on sync
behind the inputs).  Warm-up + filler matmuls on a memset tile keep the
PE busy through the DMA-bound preamble so HAM never throttles the clock.

Precision: enc / dec / W_a / softmax stay fp32r (the exp() amplifies
absolute score error, so the score path needs the 11-bit mantissa);
W_c1 / W_c2 are bf16 *stationary/moving weight* operands only, and the
output is written bf16 (host upcasts) - each adds ~1e-3 rel err against
a 2e-2 budget while cutting HBM traffic 30.4 -> 22.1 MB.

Data parallel over batch: 8 batches per core x 8 cores.
"""

import numpy as np
import ml_dtypes

import concourse.bass as bass
import concourse.bacc as bacc
import concourse.mybir as mybir
import concourse.tile as tile
from concourse.bass_utils import run_bass_kernel_spmd

# Problem constants (hardcoded per harness contract).
N, L, H, Q = 64, 258, 1024, 256
WINDOW = 16.0
DEV_POW = 128.0
NCORES = 8
B = N // NCORES  # batches per core
W = 33           # window width (positions that can survive the mask)
HC = H // 128    # h-chunks of 128 (PE contraction tiles)
F32 = mybir.dt.float32
F32R = mybir.dt.float32r
BF16 = mybir.dt.bfloat16
AF = mybir.ActivationFunctionType

# exp is computed as t = exp(s/4 + bias); bias = LOG_ALPHA keeps the
# column-sum T = sum_j t below fp32 max.  alpha cancels in w = t/T.
LOG_ALPHA = -4.8520302  # -7*ln(2)
MASK_BIAS = -10000.0    # exp(<= -9900) == 0 in fp32

OUT_NAME = "outT"


def build_nc() -> bass.Bass:
    nc = bacc.Bacc()
    enc_wT = nc.declare_dram_parameter("enc_wT", [H, B * W], BF16, isOutput=False)
    dec_hT = nc.declare_dram_parameter("dec_hT", [H, B * Q], BF16, isOutput=False)
    W_aT = nc.declare_dram_parameter("W_aT", [H, H], BF16, isOutput=False)
    W_c1T = nc.declare_dram_parameter("W_c1T", [H, H], BF16, isOutput=False)
    Wc2P = nc.declare_dram_parameter("Wc2P", [128, HC * HC * 128], BF16, isOutput=False)
    biasT = nc.declare_dram_parameter("biasT", [W, B], F32, isOutput=False)
    gPackT = nc.declare_dram_parameter("gPackT", [3 * W, 3], F32, isOutput=False)
    onesD = nc.declare_dram_parameter("onesD", [W, W], F32R, isOutput=False)
    outT = nc.declare_dram_parameter(OUT_NAME, [H, B * Q], BF16, isOutput=True)

    enc_r = enc_wT[:, :].rearrange("(c p) m -> p c m", p=128)
    WaT_r = W_aT[:, :].rearrange("(c p) m -> p c m", p=128)
    Wc1_r = W_c1T[:, :].rearrange("(c p) m -> p c m", p=128)
    Wc2_r = Wc2P[:, :].rearrange("p (o c m) -> p o c m", o=HC, c=HC)
    dec_r = dec_hT[:, :].rearrange("(c p) (n q) -> p c n q", p=128, q=Q)
    outT_r = outT[:, :].rearrange("(o p) m -> p o m", p=128)

    with tile.TileContext(nc) as tc:
        with (
            tc.tile_pool(name="const", bufs=1) as cpool,
            tc.tile_pool(name="sm", bufs=1) as sm_pool,
            # one slot per v-group evacuation: the gpsimd scatters that read
            # vst tiles drain at ~25 GB/s, and a 2-buf ring would WAR-block
            # the DVE queue (and everything behind it) on scatter completion
            tc.tile_pool(name="vstp", bufs=6) as vstp,
            tc.tile_pool(name="outp", bufs=12) as outp,
            tc.tile_pool(name="psA", bufs=2, space="PSUM") as psA,
            tc.tile_pool(name="psW", bufs=1, space="PSUM") as psW,
            tc.tile_pool(name="psG", bufs=5, space="PSUM") as psG,
        ):
            # ---------------- resident tensors ----------------
            # ALL matmul inputs are bf16: the walrus verifier forbids mixing
            # f32r with bf16 in one matmul, so score path and GEMM flip
            # together.  Simulated end-to-end rel err 8.3e-3 vs 2e-2 budget.
            enc_sb = cpool.tile([128, HC, B * W], BF16)
            WaT_sb = cpool.tile([128, HC, H], BF16)
            Wc1_sb = cpool.tile([128, HC, H], BF16)
            Wc2_sb = cpool.tile([128, HC, HC, 128], BF16)
            dec_sb = cpool.tile([128, HC, B, Q], BF16)
            uT_sb = cpool.tile([128, HC, B * W], BF16)
            v_sb = cpool.tile([W, B, H], BF16)
            bias_sb = cpool.tile([W, B], F32)
            gpack_sb = cpool.tile([3 * W, 3], F32)
            ones_sb = cpool.tile([W, W], F32R)
            wm_src = cpool.tile([128, 512], BF16)

            # warm-up / filler matmuls run on a memset tile so they have NO
            # DMA dependency: the PE starts ~6us in (right after the
            # framework's engine-rendezvous preamble) and trips the HAM
            # un-throttle before the first real matmul.  Fillers between
            # DMA-paced chunks keep the PE duty cycle high so HAM never
            # re-throttles mid-kernel.  wm lives in its own 1-buf pool so
            # late fillers never alias a recycled psG slot.
            nc.vector.memset(wm_src, 1.0)
            wm = psW.tile([128, 512], F32, tag="W", name="warm")

            def filler(n):
                for _ in range(n):
                    nc.tensor.matmul(
                        wm, lhsT=wm_src[:, 0:128], rhs=wm_src,
                        start=True, stop=True,
                    )

            # ---------------- DMA schedule ----------------
            # sync ring carries every input load in priority order; the
            # issue order IS the schedule.  scalar ring takes the tiny
            # consts so sync starts on enc immediately.
            # tiny dummy transfer warms the gpsimd SWDGE descriptor ring so
            # the v-scatters it carries later don't eat the cold-start
            scr = cpool.tile([1, 1], F32)
            scr2 = cpool.tile([1, 1], F32)
            nc.gpsimd.dma_start(out=scr, in_=biasT[0:1, 0:1])
            # same for the sync ring: absorb the ~4us first-DMA cold-start
            # before the enc chunk the warm-up matmuls wait on
            nc.sync.dma_start(out=scr2, in_=biasT[0:1, 0:1])

            nc.scalar.dma_start(out=bias_sb, in_=biasT[:, :])
            nc.scalar.dma_start(out=gpack_sb, in_=gPackT[:, :])
            nc.scalar.dma_start(out=ones_sb, in_=onesD[:, :])

            # enc and Wa interleaved in 2-chunk superchunks: the kc-outer u
            # phase consumes (enc k, Wa k) pairs as they land, and fewer,
            # bigger DMAs amortize the ~2us per-transfer completion latency
            # that otherwise staggers the early chunk semaphores
            for kc2 in range(HC // 2):
                k0 = 2 * kc2
                nc.sync.dma_start(out=enc_sb[:, k0:k0 + 2, :], in_=enc_r[:, k0:k0 + 2, :])
                nc.sync.dma_start(out=WaT_sb[:, k0:k0 + 2, :], in_=WaT_r[:, k0:k0 + 2, :])
            for i in range(4):
                nc.sync.dma_start(
                    out=Wc1_sb[:, 2 * i:2 * i + 2, :], in_=Wc1_r[:, 2 * i:2 * i + 2, :]
                )
            for n in range(2):
                nc.sync.dma_start(out=dec_sb[:, :, n, :], in_=dec_r[:, :, n, :])
            for o in range(HC):
                nc.sync.dma_start(out=Wc2_sb[:, o, :, :], in_=Wc2_r[:, o, :, :])
            for n in range(2, B):
                nc.sync.dma_start(out=dec_sb[:, :, n, :], in_=dec_r[:, :, n, :])

            # ---------------- PE warm-up (no DMA dependency) ----------------
            filler(7)

            # ---------------- u phase (kc-outer, chunk-paced) ----------------
            for half in range(2):
                hcs = range(4 * half, 4 * half + 4)
                pus = {
                    hc: psG.tile([128, B * W], F32, tag="G", name=f"pu{hc}")
                    for hc in hcs
                }
                for kc in range(HC):
                    for hc in hcs:
                        nc.tensor.matmul(
                            pus[hc],
                            lhsT=WaT_sb[:, kc, hc * 128:(hc + 1) * 128],
                            rhs=enc_sb[:, kc, :],
                            start=(kc == 0),
                            stop=(kc == HC - 1),
                        )
                    # half 0 consumes (enc, Wa) chunk pairs as they land
                    # (~1us apart vs ~0.45us of real matmuls): pad the wait;
                    # the DMA queue ramps slowly so early chunks need more
                    if half == 0 and kc < HC - 1:
                        filler(3 if kc < 3 else 2)
                for hc in hcs:
                    nc.scalar.copy(out=uT_sb[:, hc, :], in_=pus[hc])

            # ---------------- v phase ----------------
            GROUPS = [(0, 99), (99, 99), (198, 66)]

            def v_group(gi):
                g0, glen = GROUPS[gi]
                for nt in range(2):
                    pv = psG.tile([128, 512], F32, tag="G", name=f"pv{nt}_{gi}")
                    for kc in range(HC):
                        nc.tensor.matmul(
                            pv[:glen, :],
                            lhsT=enc_sb[:, kc, g0:g0 + glen],
                            rhs=Wc1_sb[:, kc, nt * 512:(nt + 1) * 512],
                            start=(kc == 0),
                            stop=(kc == HC - 1),
                        )
                        # first group's nt=0 pass paces the Wc1 chunk arrivals
                        if gi == 0 and nt == 0 and kc < HC - 1:
                            filler(1)
                    vst = vstp.tile([128, 512], BF16, tag="vst", name=f"vst{nt}_{gi}")
                    # evacuate + fold the gaussian in one op
                    nc.vector.tensor_scalar_mul(
                        vst[:glen, :], pv[:glen, :], gpack_sb[:glen, gi:gi + 1]
                    )
                    # scatters ride the gpsimd ring ONLY: its completion
                    # semaphore lanes are disjoint from the sync/scalar HWDGE
                    # lanes, so a slow 33-row scatter can never head-of-line
                    # block the main input stream's issue.
                    for off in range(glen // W):
                        n = gi * 3 + off
                        nc.gpsimd.dma_start(
                            out=v_sb[:, n, nt * 512:(nt + 1) * 512],
                            in_=vst[off * W:(off + 1) * W, :],
                        )

            # ---------------- softmax chains (pair-granular) ----------------
            # the renorm steps (column-sum matmul + reciprocal + muls) run on
            # a [33, 512] two-batch tile: half the chain matmuls / DVE ops /
            # cross-engine waits of per-batch chains.
            t_tiles = {}
            sc_state = {}

            def sc1(n):
                ps = psA.tile([W, Q], F32, tag="A", name=f"ps{n}")
                for hc in range(HC):
                    nc.tensor.matmul(
                        ps,
                        lhsT=uT_sb[:, hc, n * W:(n + 1) * W],
                        rhs=dec_sb[:, hc, n, :],
                        start=(hc == 0),
                        stop=(hc == HC - 1),
                    )
                pr, half = divmod(n, 2)
                if half == 0:
                    sc_state[pr] = sm_pool.tile(
                        [W, 2 * Q], F32R, tag="t", bufs=2, name=f"t{pr}"
                    )
                t = sc_state[pr]
                nc.scalar.activation(
                    out=t[:, half * Q:(half + 1) * Q], in_=ps, func=AF.Exp,
                    bias=bias_sb[:, n:n + 1], scale=0.25,
                )

            def sc2p(pr):
                t = sc_state[pr]
                pT = psA.tile([W, 2 * Q], F32, tag="A", name=f"pT{pr}")
                nc.tensor.matmul(pT, lhsT=ones_sb[:], rhs=t, start=True, stop=True)
                rT = sm_pool.tile([W, 2 * Q], F32, tag="r", bufs=2, name=f"rT{pr}")
                nc.vector.reciprocal_approx_fast(out=rT, in_=pT)
                nc.vector.tensor_mul(t, t, rT)
                nc.vector.tensor_mul(t, t, t)
                nc.vector.tensor_mul(t, t, t)

            def sc3p(pr):
                t = sc_state.pop(pr)
                pZ = psA.tile([W, 2 * Q], F32, tag="A", name=f"pZ{pr}")
                nc.tensor.matmul(pZ, lhsT=ones_sb[:], rhs=t, start=True, stop=True)
                rZ = sm_pool.tile([W, 2 * Q], F32, tag="r", bufs=2, name=f"rZ{pr}")
                nc.vector.reciprocal_approx_fast(out=rZ, in_=pZ)
                # final normalize writes the bf16 copy the ctx matmuls consume
                t_bf = sm_pool.tile([W, 2 * Q], BF16, tag="tb", bufs=4, name=f"tb{pr}")
                nc.vector.tensor_mul(t_bf, t, rZ)
                t_tiles[pr] = t_bf

            # ---------------- output GEMM (transposed, pipelined) ----------
            queue = []

            def unit(p, o):
                # one full-width matmul per k: a second start=True to the
                # same PSUM bank resets the whole bank, so the k=0 write
                # must cover all 512 columns at once.
                po = psG.tile([128, 512], F32, tag="G", name=f"po{p}_{o}")
                for k in range(HC):
                    nc.tensor.matmul(
                        po,
                        lhsT=Wc2_sb[:, o, k, :],
                        rhs=dec_sb[:, k, 2 * p:2 * p + 2, :],
                        start=(k == 0),
                        stop=False,
                    )
                queue.append((p, o, po))

            def flush_one():
                p, o, po = queue.pop(0)
                for i in range(2):
                    b = 2 * p + i
                    nc.tensor.matmul(
                        po[:, i * 256:(i + 1) * 256],
                        lhsT=v_sb[:, b, o * 128:(o + 1) * 128],
                        rhs=t_tiles[p][:, i * Q:(i + 1) * Q],
                        start=False,
                        stop=True,
                    )
                oT = outp.tile([128, 512], BF16, tag="o", name=f"oT{p}_{o}")
                # all stores ride the sync ring: issued (in program order)
                # after every input load, they queue behind the input stream
                # and drain at full queue rate with no lane interference.
                # outp is deep enough (12 bufs) that slots recycle without
                # ever gating the compute pipeline on a store completion.
                if p == 3 and o >= HC - 2:
                    # last two units: evacuate in halves so each store issues
                    # while the next half's tanh still runs (shortens the
                    # serial last-matmul -> last-store-complete tail)
                    for i in range(2):
                        nc.scalar.activation(
                            out=oT[:, i * 256:(i + 1) * 256],
                            in_=po[:, i * 256:(i + 1) * 256], func=AF.Tanh,
                        )
                        nc.sync.dma_start(
                            out=outT_r[:, o, p * 512 + i * 256:p * 512 + (i + 1) * 256],
                            in_=oT[:, i * 256:(i + 1) * 256],
                        )
                else:
                    nc.scalar.activation(out=oT, in_=po, func=AF.Tanh)
                    nc.sync.dma_start(out=outT_r[:, o, p * 512:(p + 1) * 512], in_=oT)

            # interleave: v groups + first two score chains before the GEMM.
            # Each score-chain step costs ~1-2us of cross-engine latency
            # (PE -> exp on scalar -> reciprocal+muls on DVE -> PE); the PE
            # queue is in-order, so a GEMM unit (~1.7us of independent
            # matmuls) is slotted between consecutive steps to keep the PE
            # streaming while the chain percolates (dec b0/b1 + Wc2[0] have
            # landed by the time the PE gets here).
            v_group(0)
            v_group(1)
            sc1(0)
            v_group(2)
            sc1(1)
            sc2p(0)

            # score-chain emission points: pair p's batches are scored
            # during pair p-1 / early pair p (dec lands long before, so the
            # chains are paced purely by PE/DVE runway: one unit (~2.1us)
            # between consecutive steps keeps every cross-engine hop hidden).
            actions = {}
            for p in range(1, 4):
                a, b = 2 * p, 2 * p + 1
                actions[(p - 1, 6)] = [lambda a=a: sc1(a)]
                actions[(p - 1, 7)] = [lambda b=b: sc1(b)]
                actions[(p, 0)] = [lambda p=p: sc2p(p)]
                actions[(p, 2)] = [lambda p=p: sc3p(p)]

            unit(0, 0)
            sc3p(0)
            unit(0, 1)
            unit(0, 2)
            unit(0, 3)

            def can_flush():
                # a unit can only flush after its pair's softmax chain has
                # been emitted (sc3p(p) lands at action point (p,2))
                return bool(queue) and queue[0][0] in t_tiles

            for p in range(4):
                for o in range(4 if p == 0 else 0, HC):
                    unit(p, o)
                    for act in actions.get((p, o), ()):
                        act()
                    while len(queue) > 4 and can_flush():
                        flush_one()
                    # drain the pipeline through the last pair so the tail
                    # is just the final unit's ctx+tanh+store chain
                    if p == 3 and o >= 1 and can_flush():
                        flush_one()
                    if p == 3 and o >= 4 and can_flush():
                        flush_one()
            while queue:
                flush_one()
    nc.compile()
    return nc


def round_f32r(a: np.ndarray) -> np.ndarray:
    """Round fp32 to fp32r (TF32-like: 11-bit mantissa, low 12 bits zero),
    round-to-nearest-even.  This is what the PE consumes in fp32r mode."""
    u = np.ascontiguousarray(a, dtype=np.float32).view(np.uint32)
    lsb = (u >> np.uint32(12)) & np.uint32(1)
    u = (u + np.uint32(0x7FF) + lsb) & np.uint32(0xFFFFF000)
    return u.view(np.float32)


def prepare_in_maps(inputs: dict) -> list[dict]:
    enc = np.asarray(inputs["encoder_outputs"], dtype=np.float32)
    dec = np.asarray(inputs["decoder_h_t"], dtype=np.float32)
    src_len = np.asarray(inputs["src_len"], dtype=np.int32)
    p_t = np.asarray(inputs["p_t"], dtype=np.float32)
    W_a = np.asarray(inputs["W_a"], dtype=np.float32)
    W_c = np.asarray(inputs["W_c"], dtype=np.float32)

    # Window bounds, computed with the same fp32 ops as the reference.
    attn_start = np.maximum(p_t - np.float32(WINDOW), np.float32(0.0))
    attn_end = np.minimum(p_t + np.float32(WINDOW), src_len.astype(np.float32))
    s = np.ceil(attn_start).astype(np.int64)
    s = np.minimum(s, L - W)  # keep the 33-slice in bounds
    idx = s[:, None] + np.arange(W)[None, :]
    idxf = idx.astype(np.float32)
    mask = (idxf < attn_start[:, None]) | (idxf > attn_end[:, None])
    bias = np.where(mask, np.float32(MASK_BIAS), np.float32(LOG_ALPHA)).astype(np.float32)
    g = np.exp(-((idxf - p_t[:, None]) ** 2) / np.float32(DEV_POW)).astype(np.float32)

    enc_w = enc[np.arange(N)[:, None], idx, :]               # [N, W, H]
    W_aT = W_a.T.astype(ml_dtypes.bfloat16)
    W_c1T = W_c[:, :H].T.astype(ml_dtypes.bfloat16)
    # W_c2T packed o-major: [HC(o), 128(o_in)] blocks contiguous per
    # partition so each o-chunk is one dense DMA.  bf16: halves the HBM
    # traffic AND halves its LDWEIGHTS cost (FWL needs a non-fp32 dtype).
    W_c2T = W_c[:, H:].T.astype(ml_dtypes.bfloat16)          # [H(h), H(h')]
    Wc2P = np.ascontiguousarray(
        W_c2T.reshape(HC, 128, HC, 128).transpose(1, 2, 0, 3).reshape(128, HC * HC * 128)
    )

    in_maps = []
    for c in range(NCORES):
        bs = slice(c * B, (c + 1) * B)
        gc = g[bs]  # [B, W]
        gpack = np.zeros((3 * W, 3), dtype=np.float32)
        for n in range(B):
            gi, off = divmod(n, 3)
            gpack[off * W:(off + 1) * W, gi] = gc[n]
        in_maps.append({
            "enc_wT": np.ascontiguousarray(
                enc_w[bs].transpose(2, 0, 1).reshape(H, B * W)
            ).astype(ml_dtypes.bfloat16),
            "dec_hT": np.ascontiguousarray(
                dec[bs].transpose(2, 0, 1).reshape(H, B * Q)
            ).astype(ml_dtypes.bfloat16),
            "W_aT": W_aT,
            "W_c1T": W_c1T,
            "Wc2P": Wc2P,
            "biasT": np.ascontiguousarray(bias[bs].T),
            "onesD": np.ones((W, W), dtype=np.float32),
            "gPackT": gpack,
        })
    return in_maps


def assemble(results) -> np.ndarray:
    """[H, B*Q] bf16 per core -> full [N, Q, H] f32."""
    outs = [
        np.asarray(results[c][OUT_NAME]).astype(np.float32).T.reshape(B, Q, H)
        for c in range(NCORES)
    ]
    return np.concatenate(outs, axis=0)


_NC = None


def get_nc() -> bass.Bass:
    global _NC
    if _NC is None:
        _NC = build_nc()
    return _NC


def kernel(**inputs) -> np.ndarray:
    nc = get_nc()
    in_maps = prepare_in_maps(inputs)
    res = run_bass_kernel_spmd(nc, in_maps, list(range(NCORES)))
    return assemble(res.results)



# revision 38
# speedup vs baseline: 1.0045x; 1.0045x over previous
"""Trainium2 Bass kernel for Luong local-p sparse attention.

Math (per batch n, full shapes N=64, L=258, H=1024, Q=256):
    score = (h_t @ W_a) @ enc^T           masked to window [p_t-16, p_t+16]
    align = softmax(score) * gauss(p_t)
    out   = tanh([align @ enc, h_t] @ W_c^T)

Only a 33-wide window of enc survives the mask, so the kernel gathers
windows host-side and pushes W_a / W_c[:, :H] through the 33-wide side:
    u  = W_a-transform of window   (uT[h', (n,j)] = sum_h W_aT[h,h'] enc_w[(n,j),h])
    s  = uT^T-partial scores       (score^T[j, q] = sum_h' uT[h',j] h_t[q,h'])
    softmax over j (33 rows) j-major with a 4th-power renormalization
    v  = W_c1-transform of window  (v[(n,j), h'] = sum_h enc_w[(n,j),h] W_c1T[h,h'])
    outT[h', (n,q)] = tanh(sum_h W_c2T[h,h'] dec[h,(n,q)] + sum_j v[j,h'] t[j,q])

The output GEMM runs TRANSPOSED (outT layout [H, B*Q]): stationary W_c2
chunks are shared across batches, the per-batch ctx matmuls (contraction
over the 33 window rows) accumulate into the same PSUM tiles, and tanh +
stores pipeline per 128-row output chunk (no big tail).

DMA priority order: enc -> W_a -> W_c1 -> dec b0,b1 -> W_c2 -> dec b2..b7,
all on the sync ring (queue 1 alone sustains ~390 GB/s; the 8 HWDGE
completion-semaphore lanes are shared sync<->scalar, so v-window scatters
ride the gpsimd ring's disjoint lanes and output stores queue on sync
behind the inputs).  Warm-up + filler matmuls on a memset tile keep the
PE busy through the DMA-bound preamble so HAM never throttles the clock.

Precision: enc / dec / W_a / softmax stay fp32r (the exp() amplifies
absolute score error, so the score path needs the 11-bit mantissa);
W_c1 / W_c2 are bf16 *stationary/moving weight* operands only, and the
output is written bf16 (host upcasts) - each adds ~1e-3 rel err against
a 2e-2 budget while cutting HBM traffic 30.4 -> 22.1 MB.

Data parallel over batch: 8 batches per core x 8 cores.
"""

import numpy as np
import ml_dtypes

import concourse.bass as bass
import concourse.bacc as bacc
import concourse.mybir as mybir
import concourse.tile as tile
from concourse.bass_utils import run_bass_kernel_spmd

# Problem constants (hardcoded per harness contract).
N, L, H, Q = 64, 258, 1024, 256
WINDOW = 16.0
DEV_POW = 128.0
NCORES = 8
B = N // NCORES  # batches per core
W = 33           # window width (positions that can survive the mask)
HC = H // 128    # h-chunks of 128 (PE contraction tiles)
F32 = mybir.dt.float32
F32R = mybir.dt.float32r
BF16 = mybir.dt.bfloat16
AF = mybir.ActivationFunctionType

# exp is computed as t = exp(s/4 + bias); bias = LOG_ALPHA keeps the
# column-sum T = sum_j t below fp32 max.  alpha cancels in w = t/T.
LOG_ALPHA = -4.8520302  # -7*ln(2)
MASK_BIAS = -10000.0    # exp(<= -9900) == 0 in fp32

OUT_NAME = "outT"


def build_nc() -> bass.Bass:
    nc = bacc.Bacc()
    enc_wT = nc.declare_dram_parameter("enc_wT", [H, B * W], BF16, isOutput=False)
    dec_hT = nc.declare_dram_parameter("dec_hT", [H, B * Q], BF16, isOutput=False)
    W_aT = nc.declare_dram_parameter("W_aT", [H, H], BF16, isOutput=False)
    W_c1T = nc.declare_dram_parameter("W_c1T", [H, H], BF16, isOutput=False)
    Wc2P = nc.declare_dram_parameter("Wc2P", [128, HC * HC * 128], BF16, isOutput=False)
    biasT = nc.declare_dram_parameter("biasT", [W, B], F32, isOutput=False)
    gPackT = nc.declare_dram_parameter("gPackT", [3 * W, 3], F32, isOutput=False)
    onesD = nc.declare_dram_parameter("onesD", [W, W], F32R, isOutput=False)
    outT = nc.declare_dram_parameter(OUT_NAME, [H, B * Q], BF16, isOutput=True)

    enc_r = enc_wT[:, :].rearrange("(c p) m -> p c m", p=128)
    WaT_r = W_aT[:, :].rearrange("(c p) m -> p c m", p=128)
    Wc1_r = W_c1T[:, :].rearrange("(c p) m -> p c m", p=128)
    Wc2_r = Wc2P[:, :].rearrange("p (o c m) -> p o c m", o=HC, c=HC)
    dec_r = dec_hT[:, :].rearrange("(c p) (n q) -> p c n q", p=128, q=Q)
    outT_r = outT[:, :].rearrange("(o p) m -> p o m", p=128)

    with tile.TileContext(nc) as tc:
        with (
            tc.tile_pool(name="const", bufs=1) as cpool,
            tc.tile_pool(name="sm", bufs=1) as sm_pool,
            # one slot per v-group evacuation: the gpsimd scatters that read
            # vst tiles drain at ~25 GB/s, and a 2-buf ring would WAR-block
            # the DVE queue (and everything behind it) on scatter completion
            tc.tile_pool(name="vstp", bufs=6) as vstp,
            tc.tile_pool(name="outp", bufs=12) as outp,
            tc.tile_pool(name="psA", bufs=2, space="PSUM") as psA,
            tc.tile_pool(name="psW", bufs=1, space="PSUM") as psW,
            tc.tile_pool(name="psG", bufs=5, space="PSUM") as psG,
        ):
            # ---------------- resident tensors ----------------
            # ALL matmul inputs are bf16: the walrus verifier forbids mixing
            # f32r with bf16 in one matmul, so score path and GEMM flip
            # together.  Simulated end-to-end rel err 8.3e-3 vs 2e-2 budget.
            enc_sb = cpool.tile([128, HC, B * W], BF16)
            WaT_sb = cpool.tile([128, HC, H], BF16)
            Wc1_sb = cpool.tile([128, HC, H], BF16)
            Wc2_sb = cpool.tile([128, HC, HC, 128], BF16)
            dec_sb = cpool.tile([128, HC, B, Q], BF16)
            uT_sb = cpool.tile([128, HC, B * W], BF16)
            v_sb = cpool.tile([W, B, H], BF16)
            bias_sb = cpool.tile([W, B], F32)
            gpack_sb = cpool.tile([3 * W, 3], F32)
            ones_sb = cpool.tile([W, W], F32R)
            wm_src = cpool.tile([128, 512], BF16)

            # warm-up / filler matmuls run on a memset tile so they have NO
            # DMA dependency: the PE starts ~6us in (right after the
            # framework's engine-rendezvous preamble) and trips the HAM
            # un-throttle before the first real matmul.  Fillers between
            # DMA-paced chunks keep the PE duty cycle high so HAM never
            # re-throttles mid-kernel.  wm lives in its own 1-buf pool so
            # late fillers never alias a recycled psG slot.
            nc.vector.memset(wm_src, 1.0)
            wm = psW.tile([128, 512], F32, tag="W", name="warm")

            def filler(n):
                for _ in range(n):
                    nc.tensor.matmul(
                        wm, lhsT=wm_src[:, 0:128], rhs=wm_src,
                        start=True, stop=True,
                    )

            # ---------------- DMA schedule ----------------
            # sync ring carries every input load in priority order; the
            # issue order IS the schedule.  scalar ring takes the tiny
            # consts so sync starts on enc immediately.
            # tiny dummy transfer warms the gpsimd SWDGE descriptor ring so
            # the v-scatters it carries later don't eat the cold-start
            scr = cpool.tile([1, 1], F32)
            scr2 = cpool.tile([1, 1], F32)
            nc.gpsimd.dma_start(out=scr, in_=biasT[0:1, 0:1])
            # same for the sync ring: absorb the ~4us first-DMA cold-start
            # before the enc chunk the warm-up matmuls wait on
            nc.sync.dma_start(out=scr2, in_=biasT[0:1, 0:1])

            nc.scalar.dma_start(out=bias_sb, in_=biasT[:, :])
            nc.scalar.dma_start(out=gpack_sb, in_=gPackT[:, :])
            nc.scalar.dma_start(out=ones_sb, in_=onesD[:, :])

            # enc and Wa interleaved in 2-chunk superchunks: the kc-outer u
            # phase consumes (enc k, Wa k) pairs as they land, and fewer,
            # bigger DMAs amortize the ~2us per-transfer completion latency
            # that otherwise staggers the early chunk semaphores
            for kc2 in range(HC // 2):
                k0 = 2 * kc2
                nc.sync.dma_start(out=enc_sb[:, k0:k0 + 2, :], in_=enc_r[:, k0:k0 + 2, :])
                nc.sync.dma_start(out=WaT_sb[:, k0:k0 + 2, :], in_=WaT_r[:, k0:k0 + 2, :])
            for i in range(4):
                nc.sync.dma_start(
                    out=Wc1_sb[:, 2 * i:2 * i + 2, :], in_=Wc1_r[:, 2 * i:2 * i + 2, :]
                )
            for n in range(2):
                nc.sync.dma_start(out=dec_sb[:, :, n, :], in_=dec_r[:, :, n, :])
            for o in range(HC):
                nc.sync.dma_start(out=Wc2_sb[:, o, :, :], in_=Wc2_r[:, o, :, :])
            for n in range(2, B):
                nc.sync.dma_start(out=dec_sb[:, :, n, :], in_=dec_r[:, :, n, :])

            # ---------------- PE warm-up (no DMA dependency) ----------------
            filler(7)

            # ---------------- u phase (kc-outer, chunk-paced) ----------------
            for half in range(2):
                hcs = range(4 * half, 4 * half + 4)
                pus = {
                    hc: psG.tile([128, B * W], F32, tag="G", name=f"pu{hc}")
                    for hc in hcs
                }
                for kc in range(HC):
                    for hc in hcs:
                        nc.tensor.matmul(
                            pus[hc],
                            lhsT=WaT_sb[:, kc, hc * 128:(hc + 1) * 128],
                            rhs=enc_sb[:, kc, :],
                            start=(kc == 0),
                            stop=(kc == HC - 1),
                        )
                    # half 0 consumes (enc, Wa) chunk pairs as they land
                    # (~1us apart vs ~0.45us of real matmuls): pad the wait;
                    # the DMA queue ramps slowly so early chunks need more
                    if half == 0 and kc < HC - 1:
                        filler(3 if kc < 3 else 2)
                for hc in hcs:
                    nc.scalar.copy(out=uT_sb[:, hc, :], in_=pus[hc])

            # ---------------- v phase ----------------
            GROUPS = [(0, 99), (99, 99), (198, 66)]

            def v_group(gi):
                g0, glen = GROUPS[gi]
                for nt in range(2):
                    pv = psG.tile([128, 512], F32, tag="G", name=f"pv{nt}_{gi}")
                    for kc in range(HC):
                        nc.tensor.matmul(
                            pv[:glen, :],
                            lhsT=enc_sb[:, kc, g0:g0 + glen],
                            rhs=Wc1_sb[:, kc, nt * 512:(nt + 1) * 512],
                            start=(kc == 0),
                            stop=(kc == HC - 1),
                        )
                        # first group's nt=0 pass paces the Wc1 chunk arrivals
                        if gi == 0 and nt == 0 and kc < HC - 1:
                            filler(1)
                    vst = vstp.tile([128, 512], BF16, tag="vst", name=f"vst{nt}_{gi}")
                    # evacuate + fold the gaussian in one op
                    nc.vector.tensor_scalar_mul(
                        vst[:glen, :], pv[:glen, :], gpack_sb[:glen, gi:gi + 1]
                    )
                    # scatters ride the gpsimd ring ONLY: its completion
                    # semaphore lanes are disjoint from the sync/scalar HWDGE
                    # lanes, so a slow 33-row scatter can never head-of-line
                    # block the main input stream's issue.
                    for off in range(glen // W):
                        n = gi * 3 + off
                        nc.gpsimd.dma_start(
                            out=v_sb[:, n, nt * 512:(nt + 1) * 512],
                            in_=vst[off * W:(off + 1) * W, :],
                        )

            # ---------------- softmax chains (split into 3 PE steps) ------
            t_tiles = {}
            sc_state = {}

            def sc1(n):
                ps = psA.tile([W, Q], F32, tag="A", name=f"ps{n}")
                for hc in range(HC):
                    nc.tensor.matmul(
                        ps,
                        lhsT=uT_sb[:, hc, n * W:(n + 1) * W],
                        rhs=dec_sb[:, hc, n, :],
                        start=(hc == 0),
                        stop=(hc == HC - 1),
                    )
                t = sm_pool.tile([W, Q], F32R, tag="t", bufs=3, name=f"t{n}")
                nc.scalar.activation(
                    out=t, in_=ps, func=AF.Exp, bias=bias_sb[:, n:n + 1], scale=0.25
                )
                sc_state[n] = t

            def sc2(n):
                t = sc_state[n]
                pT = psA.tile([W, Q], F32, tag="A", name=f"pT{n}")
                nc.tensor.matmul(pT, lhsT=ones_sb[:], rhs=t, start=True, stop=True)
                rT = sm_pool.tile([W, Q], F32, tag="r", bufs=2, name=f"rT{n}")
                nc.vector.reciprocal_approx_fast(out=rT, in_=pT)
                nc.vector.tensor_mul(t, t, rT)
                nc.vector.tensor_mul(t, t, t)
                nc.vector.tensor_mul(t, t, t)

            def sc3(n):
                t = sc_state.pop(n)
                pZ = psA.tile([W, Q], F32, tag="A", name=f"pZ{n}")
                nc.tensor.matmul(pZ, lhsT=ones_sb[:], rhs=t, start=True, stop=True)
                rZ = sm_pool.tile([W, Q], F32, tag="r", bufs=2, name=f"rZ{n}")
                nc.vector.reciprocal_approx_fast(out=rZ, in_=pZ)
                # final normalize writes the bf16 copy the ctx matmuls consume
                t_bf = sm_pool.tile([W, Q], BF16, tag="tb", bufs=8, name=f"tb{n}")
                nc.vector.tensor_mul(t_bf, t, rZ)
                t_tiles[n] = t_bf

            # ---------------- output GEMM (transposed, pipelined) ----------
            queue = []

            def unit(p, o):
                # one full-width matmul per k: a second start=True to the
                # same PSUM bank resets the whole bank, so the k=0 write
                # must cover all 512 columns at once.
                po = psG.tile([128, 512], F32, tag="G", name=f"po{p}_{o}")
                for k in range(HC):
                    nc.tensor.matmul(
                        po,
                        lhsT=Wc2_sb[:, o, k, :],
                        rhs=dec_sb[:, k, 2 * p:2 * p + 2, :],
                        start=(k == 0),
                        stop=False,
                    )
                queue.append((p, o, po))

            def flush_one():
                p, o, po = queue.pop(0)
                for i in range(2):
                    b = 2 * p + i
                    nc.tensor.matmul(
                        po[:, i * 256:(i + 1) * 256],
                        lhsT=v_sb[:, b, o * 128:(o + 1) * 128],
                        rhs=t_tiles[b],
                        start=False,
                        stop=True,
                    )
                oT = outp.tile([128, 512], BF16, tag="o", name=f"oT{p}_{o}")
                # all stores ride the sync ring: issued (in program order)
                # after every input load, they queue behind the input stream
                # and drain at full queue rate with no lane interference.
                # outp is deep enough (12 bufs) that slots recycle without
                # ever gating the compute pipeline on a store completion.
                if p == 3 and o >= HC - 2:
                    # last two units: evacuate in halves so each store issues
                    # while the next half's tanh still runs (shortens the
                    # serial last-matmul -> last-store-complete tail)
                    for i in range(2):
                        nc.scalar.activation(
                            out=oT[:, i * 256:(i + 1) * 256],
                            in_=po[:, i * 256:(i + 1) * 256], func=AF.Tanh,
                        )
                        nc.sync.dma_start(
                            out=outT_r[:, o, p * 512 + i * 256:p * 512 + (i + 1) * 256],
                            in_=oT[:, i * 256:(i + 1) * 256],
                        )
                else:
                    nc.scalar.activation(out=oT, in_=po, func=AF.Tanh)
                    nc.sync.dma_start(out=outT_r[:, o, p * 512:(p + 1) * 512], in_=oT)

            # interleave: v groups + first two score chains before the GEMM.
            # Each score-chain step costs ~1-2us of cross-engine latency
            # (PE -> exp on scalar -> reciprocal+muls on DVE -> PE); the PE
            # queue is in-order, so a GEMM unit (~1.7us of independent
            # matmuls) is slotted between consecutive steps to keep the PE
            # streaming while the chain percolates (dec b0/b1 + Wc2[0] have
            # landed by the time the PE gets here).
            v_group(0)
            v_group(1)
            sc1(0)
            v_group(2)
            sc1(1)
            sc2(0)

            # score-chain emission points: pair p's batches are scored
            # during pair p-1 / early pair p (dec lands long before, so the
            # chains are paced purely by PE/DVE runway: one unit (~2.1us)
            # between consecutive steps keeps every cross-engine hop hidden).
            actions = {}
            for p in range(1, 4):
                a, b = 2 * p, 2 * p + 1
                actions[(p - 1, 6)] = [lambda a=a: sc1(a)]
                actions[(p - 1, 7)] = [lambda b=b: sc1(b)]
                actions[(p, 0)] = [lambda a=a: sc2(a)]
                actions[(p, 1)] = [lambda b=b: sc2(b)]
                actions[(p, 2)] = [lambda a=a: sc3(a)]
                actions[(p, 3)] = [lambda b=b: sc3(b)]

            unit(0, 0)
            sc3(0)
            sc2(1)
            unit(0, 1)
            sc3(1)
            unit(0, 2)
            unit(0, 3)

            def can_flush():
                # a unit can only flush after its pair's softmax chains have
                # been emitted (sc3(2p+1) lands at action point (p,3))
                return bool(queue) and (2 * queue[0][0] + 1) in t_tiles

            for p in range(4):
                for o in range(4 if p == 0 else 0, HC):
                    unit(p, o)
                    for act in actions.get((p, o), ()):
                        act()
                    while len(queue) > 4 and can_flush():
                        flush_one()
                    # drain the pipeline through the last pair so the tail
                    # is just the final unit's ctx+tanh+store chain
                    if p == 3 and o >= 1 and can_flush():
                        flush_one()
                    if p == 3 and o >= 4 and can_flush():
                        flush_one()
            while queue:
                flush_one()
    nc.compile()
    return nc


def round_f32r(a: np.ndarray) -> np.ndarray:
    """Round fp32 to fp32r (TF32-like: 11-bit mantissa, low 12 bits zero),
    round-to-nearest-even.  This is what the PE consumes in fp32r mode."""
    u = np.ascontiguousarray(a, dtype=np.float32).view(np.uint32)
    lsb = (u >> np.uint32(12)) & np.uint32(1)
    u = (u + np.uint32(0x7FF) + lsb) & np.uint32(0xFFFFF000)
    return u.view(np.float32)


def prepare_in_maps(inputs: dict) -> list[dict]:
    enc = np.asarray(inputs["encoder_outputs"], dtype=np.float32)
    dec = np.asarray(inputs["decoder_h_t"], dtype=np.float32)
    src_len = np.asarray(inputs["src_len"], dtype=np.int32)
    p_t = np.asarray(inputs["p_t"], dtype=np.float32)
    W_a = np.asarray(inputs["W_a"], dtype=np.float32)
    W_c = np.asarray(inputs["W_c"], dtype=np.float32)

    # Window bounds, computed with the same fp32 ops as the reference.
    attn_start = np.maximum(p_t - np.float32(WINDOW), np.float32(0.0))
    attn_end = np.minimum(p_t + np.float32(WINDOW), src_len.astype(np.float32))
    s = np.ceil(attn_start).astype(np.int64)
    s = np.minimum(s, L - W)  # keep the 33-slice in bounds
    idx = s[:, None] + np.arange(W)[None, :]
    idxf = idx.astype(np.float32)
    mask = (idxf < attn_start[:, None]) | (idxf > attn_end[:, None])
    bias = np.where(mask, np.float32(MASK_BIAS), np.float32(LOG_ALPHA)).astype(np.float32)
    g = np.exp(-((idxf - p_t[:, None]) ** 2) / np.float32(DEV_POW)).astype(np.float32)

    enc_w = enc[np.arange(N)[:, None], idx, :]               # [N, W, H]
    W_aT = W_a.T.astype(ml_dtypes.bfloat16)
    W_c1T = W_c[:, :H].T.astype(ml_dtypes.bfloat16)
    # W_c2T packed o-major: [HC(o), 128(o_in)] blocks contiguous per
    # partition so each o-chunk is one dense DMA.  bf16: halves the HBM
    # traffic AND halves its LDWEIGHTS cost (FWL needs a non-fp32 dtype).
    W_c2T = W_c[:, H:].T.astype(ml_dtypes.bfloat16)          # [H(h), H(h')]
    Wc2P = np.ascontiguousarray(
        W_c2T.reshape(HC, 128, HC, 128).transpose(1, 2, 0, 3).reshape(128, HC * HC * 128)
    )

    in_maps = []
    for c in range(NCORES):
        bs = slice(c * B, (c + 1) * B)
        gc = g[bs]  # [B, W]
        gpack = np.zeros((3 * W, 3), dtype=np.float32)
        for n in range(B):
            gi, off = divmod(n, 3)
            gpack[off * W:(off + 1) * W, gi] = gc[n]
        in_maps.append({
            "enc_wT": np.ascontiguousarray(
                enc_w[bs].transpose(2, 0, 1).reshape(H, B * W)
            ).astype(ml_dtypes.bfloat16),
            "dec_hT": np.ascontiguousarray(
                dec[bs].transpose(2, 0, 1).reshape(H, B * Q)
            ).astype(ml_dtypes.bfloat16),
            "W_aT": W_aT,
            "W_c1T": W_c1T,
            "Wc2P": Wc2P,
            "biasT": np.ascontiguousarray(bias[bs].T),
            "onesD": np.ones((W, W), dtype=np.float32),
            "gPackT": gpack,
        })
    return in_maps


def assemble(results) -> np.ndarray:
    """[H, B*Q] bf16 per core -> full [N, Q, H] f32."""
    outs = [
        np.asarray(results[c][OUT_NAME]).astype(np.float32).T.reshape(B, Q, H)
        for c in range(NCORES)
    ]
    return np.concatenate(outs, axis=0)


_NC = None


def get_nc() -> bass.Bass:
    global _NC
    if _NC is None:
        _NC = build_nc()
    return _NC


def kernel(**inputs) -> np.ndarray:
    nc = get_nc()
    in_maps = prepare_in_maps(inputs)
    res = run_bass_kernel_spmd(nc, in_maps, list(range(NCORES)))
    return assemble(res.results)

